# revision 17
# baseline (speedup 1.0000x reference)
"""AttentionRNN Trainium2 kernel (v3).

Data-parallel across 8 NeuronCores on the batch axis (B=8 -> 1 sequence per
core). Everything (embedding gather, input projection, sequential RNN scan,
additive attention, output projection) runs on-device; the host only reshapes
and shards inputs and reassembles the output.

v3 architecture (from trace analysis):
 - The scan is a serial chain (4 W_hh matmuls -> tanh) at ~660ns/step; any
   ACT instruction costs >=290ns so NO attention tanh can ride under it.
   The scan phase is kept PURE: only PE-slack-sized q/k projection pieces
   (2 matmuls each) and the idle-DVE energy adds for early chunks ride along.
 - x-inject is ONE 512-wide matmul into a persistent PSUM bank; the per-step
   W_hh matmuls accumulate into their column (has_written is per-element,
   and a single start=True write covers the whole bank).
 - Energy adds for chunks 0..HOLD-1 run during the scan on the idle DVE
   (broadcast tensor_tensor) into exactly-sized held SBUF tiles; their tanh
   runs as a burst right after the scan.
 - Post-scan phase pipelines: chunk tanh (ACT) -> v-reduce (PE) -> scatter
   (gpsimd DMA), overlapped with the out-projection stream (PE). mt=0 rows
   of all vocab slabs run as soon as context half 0 exists (ring slabs are
   re-DMAed for the mt=1 pass; slabs 0..NPRE-1 persist in SBUF).

Self-contained: hardcodes all shapes; reads nothing from disk.
"""

import sys

sys.path.insert(0, "/opt/trn_rl_repo")

import numpy as np

import concourse.bacc as bacc
import concourse.mybir as mybir
import concourse.tile as tile
from concourse.bass import IndirectOffsetOnAxis
from concourse.bass_utils import run_bass_kernel_spmd
from concourse.masks import make_identity

V, E, H, B, T = 32000, 256, 256, 8, 256
P = 128
NCORE = 8
F32 = mybir.dt.float32
I32 = mybir.dt.int32
AF = mybir.ActivationFunctionType
BF16 = mybir.dt.bfloat16
FP16 = mybir.dt.float16
SCAN_NP = np.float16  # numpy dtype fed to the whh input
CH = 8  # t-chunk size in the attention energy loop
QKB = 64  # qk projection block width
NS = 1000  # free-dim slab for the output projection stream
SUB = 500  # matmul free-dim sub-chunk (one PSUM bank)
NPRE = 8  # W_out slabs prefetched from kernel start (persist for late mt=1)
HOLD = 16  # chunks whose energy adds ride the scan into held SBUF tiles


def _r2(w):
    """[2P, M] -> [P, 2, M] with w2[p, k, m] = w[k*P+p, m]"""
    return np.ascontiguousarray(w.reshape(2, P, -1).transpose(1, 0, 2))


def _col(b):
    """[2P] -> [P, 2] with c[p, k] = b[k*P+p]"""
    return np.ascontiguousarray(b.reshape(2, P).T)


def build_nc(dbg=False, zb=False):
    nc = bacc.Bacc("TRN2", target_bir_lowering=False, debug=False)

    idx_d = nc.dram_tensor("idx", [P, 2], I32, kind="ExternalInput")
    emb_d = nc.dram_tensor("emb", [V, E], F32, kind="ExternalInput")
    wih_d = nc.dram_tensor("wih", [P, 2, H], F32, kind="ExternalInput")
    whh_d = nc.dram_tensor("whh", [P, 2, H], FP16, kind="ExternalInput")
    wac_d = nc.dram_tensor("wac", [P, 2, H], FP16, kind="ExternalInput")
    wap_d = nc.dram_tensor("wap", [P, 2, H], FP16, kind="ExternalInput")
    bih_d = nc.dram_tensor("bih", [P, 2], F32, kind="ExternalInput")
    bhh_d = nc.dram_tensor("bhh", [P, 2], F32, kind="ExternalInput")
    bac_d = nc.dram_tensor("bac", [P, 2], F32, kind="ExternalInput")
    bap_d = nc.dram_tensor("bap", [P, 2], F32, kind="ExternalInput")
    v_d = nc.dram_tensor("vcol", [P, 2], FP16, kind="ExternalInput")
    mask_d = nc.dram_tensor("maskadd", [P, 2, T], F32, kind="ExternalInput")
    wout_d = nc.dram_tensor("wout", [P, 4, V], BF16, kind="ExternalInput")
    bout_d = nc.dram_tensor("bout", [1, V], BF16, kind="ExternalInput")
    ones_d = nc.dram_tensor("ones", [1, P], BF16, kind="ExternalInput")
    out_d = nc.dram_tensor("out", [T, V], FP16, kind="ExternalOutput")

    with tile.TileContext(nc) as tc:
        with tc.tile_pool(name="persist", bufs=1) as pp:
            # --- persistent SBUF state ---
            idx_sb = pp.tile([P, 2], I32)
            wih = pp.tile([P, 2, H], F32)
            whh = pp.tile([P, 2, H], FP16)
            wac = pp.tile([P, 2, H], FP16)
            wap = pp.tile([P, 2, H], FP16)
            bih = pp.tile([P, 2], F32)
            bhh = pp.tile([P, 2], F32)
            bac = pp.tile([P, 2], F32)
            bap = pp.tile([P, 2], F32)
            vcol = pp.tile([P, 2], FP16)
            maskadd = pp.tile([P, 2, T], F32)
            ident = pp.tile([P, P], F32)
            ones_row = pp.tile([1, P], BF16)
            bx = pp.tile([P, 2], F32)
            embT = pp.tile([P, 2, T], F32)  # [e_p, et, t]
            xT = pp.tile([P, 2, T], F32)  # [h_p, ht, t] = x_proj^T + b_ih + b_hh
            combT = pp.tile([P, 4, T], FP16)  # [:,0:2]=context^T, [:,2:4]=Hs^T
            ident16 = pp.tile([P, P], FP16)
            qT = pp.tile([P, 2, T], FP16)
            scores = pp.tile([P, 2, T], F32)  # [tp, tc, j], t = tc*128+tp
            ssum = pp.tile([P, 2], F32)
            srecip = pp.tile([P, 2], F32)
            alphaT = pp.tile([P, 2, T], FP16)  # [j_p, jt, t]
            hs = pp.tile([P, 2, H], FP16)  # [t_p, tc, h] (Hs, untransposed)
            combTr = pp.tile([P, 4, T], BF16)  # bf16 copy for the out-proj
            kTb = pp.tile([P, 2, T], FP16)  # fp16 copy of kT for the energy adds
            xT16 = pp.tile([P, 2, T], FP16)  # fp16 xT for the scan x-inject mm

            nc.sync.dma_start(idx_sb[:], idx_d[:])
            nc.sync.dma_start(wih[:], wih_d[:])
            nc.sync.dma_start(whh[:], whh_d[:])
            nc.sync.dma_start(wac[:], wac_d[:])
            nc.sync.dma_start(wap[:], wap_d[:])
            nc.sync.dma_start(bih[:], bih_d[:])
            nc.sync.dma_start(bhh[:], bhh_d[:])
            nc.sync.dma_start(bac[:], bac_d[:])
            nc.sync.dma_start(bap[:], bap_d[:])
            nc.sync.dma_start(vcol[:], v_d[:])
            nc.sync.dma_start(maskadd[:], mask_d[:])
            make_identity(nc, ident[:])
            nc.vector.tensor_copy(ident16[:], ident[:])
            nc.sync.dma_start(ones_row[:], ones_d[:])
            nc.vector.tensor_add(bx[:], bih[:], bhh[:])

            # W_out slab prefetch pool: NPRE tiles persist the whole kernel
            # (consumed by mt=0 early and mt=1 at the very end).
            wp_cm = tc.tile_pool(name="wpool", bufs=NPRE)
            wp = wp_cm.__enter__()
            wsl_tiles = []
            for s_ in range(NPRE):
                wsl = wp.tile([P, 4, NS], BF16, tag="wslab")
                nc.sync.dma_start(wsl[:], wout_d[:, :, s_ * NS : (s_ + 1) * NS])
                wsl_tiles.append(wsl)

            # --- phase A: embedding gather + transpose to embT [e, t] ---
            with (
                tc.tile_pool(name="pha", bufs=2) as pa,
                tc.tile_pool(name="pha_ps", bufs=2, space="PSUM") as pa_ps,
            ):
                for c in range(2):
                    emb_g = pa.tile([P, E], F32, tag="embg")
                    nc.gpsimd.indirect_dma_start(
                        out=emb_g[:],
                        out_offset=None,
                        in_=emb_d[:, :],
                        in_offset=IndirectOffsetOnAxis(ap=idx_sb[:, c : c + 1], axis=0),
                    )
                    for et_ in range(2):
                        tr_ps = pa_ps.tile([P, P], F32, tag="trps")
                        nc.tensor.transpose(
                            tr_ps[:], emb_g[:, et_ * P : (et_ + 1) * P], ident[:]
                        )
                        nc.vector.tensor_copy(
                            embT[:, et_, c * P : (c + 1) * P], tr_ps[:]
                        )

                # --- phase B: xT = (emb @ W_ih)^T + b_ih + b_hh ---
                for mt in range(2):
                    ps = pa_ps.tile([P, T], F32, tag="projps")
                    for kt in range(2):
                        nc.tensor.matmul(
                            ps[:],
                            wih[:, kt, mt * P : (mt + 1) * P],
                            embT[:, kt, :],
                            start=(kt == 0),
                            stop=(kt == 1),
                        )
                    if zb:
                        nc.scalar.activation(xT[:, mt, :], ps[:], AF.Copy)
                    else:
                        nc.scalar.activation(
                            xT[:, mt, :], ps[:], AF.Identity, bias=bx[:, mt : mt + 1]
                        )
                    nc.vector.tensor_copy(xT16[:, mt, :], xT[:, mt, :])

            # --- scan phase + post-scan attention/out-proj ---
            n_chunks = T // CH
            with (
                tc.tile_pool(name="scan_ps", bufs=1, space="PSUM") as sc_ps,
                tc.tile_pool(name="qk_ps", bufs=2, space="PSUM") as qk_ps,
                tc.tile_pool(name="ehold", bufs=1) as eh,
                tc.tile_pool(name="epool", bufs=3) as ep,
                tc.tile_pool(name="rowpool", bufs=2) as rp,
                tc.tile_pool(name="scrpool", bufs=4, space="DRAM") as scrp,
                tc.tile_pool(name="eps", bufs=1, space="PSUM") as e_ps,
                tc.tile_pool(name="bpool", bufs=2) as bp,
                tc.tile_pool(name="opool", bufs=3) as op,
                tc.tile_pool(name="pg_ps", bufs=3, space="PSUM") as pg_ps,
            ):
                qp_tiles = {}

                def emit_qk_piece(b, g):
                    """One (wi, mt) output group (2 matmuls) of qk block b."""
                    cols = slice(QKB * b, QKB * b + QKB)
                    if g == 0:
                        qp_tiles[b] = qk_ps.tile(
                            [P, 4, QKB], F32, tag="qkps", name=f"qkps{b}"
                        )
                    qp = qp_tiles[b]
                    wi, mt = divmod(g, 2)
                    w_sb = (wac, wap)[wi]
                    for kt in range(2):
                        nc.tensor.matmul(
                            qp[:, g, :],
                            w_sb[:, kt, mt * P : (mt + 1) * P],
                            combT[:, 2 + kt, cols],
                            start=(kt == 0),
                            stop=(kt == 1),
                        )

                def emit_qk_copies(b):
                    cols = slice(QKB * b, QKB * b + QKB)
                    qp = qp_tiles.pop(b)
                    for mt in range(2):
                        if zb:
                            nc.vector.tensor_copy(qT[:, mt, cols], qp[:, mt, :])
                            nc.vector.tensor_copy(kTb[:, mt, cols], qp[:, 2 + mt, :])
                        else:
                            nc.scalar.activation(
                                qT[:, mt, cols],
                                qp[:, mt, :],
                                AF.Identity,
                                bias=bac[:, mt : mt + 1],
                            )
                            nc.scalar.activation(
                                kTb[:, mt, cols],
                                qp[:, 2 + mt, :],
                                AF.Identity,
                                bias=bap[:, mt : mt + 1],
                            )

                et_tiles = {}

                def emit_chunk_energy(c):
                    """Energy adds for t-rows [8c, 8c+8) (DVE only)."""
                    t0 = c * CH
                    jcap = t0 + CH
                    w = CH * jcap
                    if c < HOLD:
                        et = eh.tile([P, 2, w], FP16, tag=f"eth{c}")
                    else:
                        et = ep.tile([P, 2, CH * T], FP16, tag="etile", name=f"et{c}")
                    et_tiles[c] = et
                    for kt in range(2):
                        etv = et[:, kt, :w].rearrange("p (c j) -> p c j", j=jcap)
                        kb = kTb[:, kt : kt + 1, :jcap].to_broadcast([P, CH, jcap])
                        qb = qT[:, kt, t0 : t0 + CH].rearrange(
                            "p (c o) -> p c o", o=1
                        ).to_broadcast([P, CH, jcap])
                        nc.vector.tensor_tensor(etv, kb, qb, mybir.AluOpType.add)

                def emit_chunk_tanh(c):
                    t0 = c * CH
                    w = CH * (t0 + CH)
                    et = et_tiles[c]
                    nc.scalar.activation(et[:, :, :w], et[:, :, :w], AF.Tanh)

                def emit_chunk_reduce(c):
                    """v-reduce + score scatter for chunk c (PE+DVE+DMA)."""
                    t0 = c * CH
                    jcap = t0 + CH
                    w = CH * jcap
                    et = et_tiles.pop(c)
                    row = rp.tile([1, CH * T], F32, tag="rowtile", name=f"rw{c}")
                    ngrp = 1
                    while (CH // ngrp) * jcap > 512:
                        ngrp *= 2
                    gw = (CH // ngrp) * jcap
                    for h_ in range((ngrp + 1) // 2):
                        gs = [g for g in (2 * h_, 2 * h_ + 1) if g < ngrp]
                        ps = e_ps.tile(
                            [1, 2, 512], F32, tag="spsum", name=f"sps{c}_{h_}"
                        )
                        for gi, g in enumerate(gs):
                            sl = slice(g * gw, (g + 1) * gw)
                            for kt in range(2):
                                nc.tensor.matmul(
                                    ps[0:1, gi, :gw],
                                    vcol[:, kt : kt + 1],
                                    et[:, kt, sl],
                                    start=(kt == 0),
                                    stop=(kt == 1),
                                )
                        rview = row[
                            :, 2 * h_ * gw : (2 * h_ + len(gs)) * gw
                        ].rearrange("p (g x) -> p g x", g=len(gs))
                        nc.vector.tensor_copy(rview, ps[:, : len(gs), :gw])
                    tc_i = t0 // P
                    tp0 = t0 % P
                    scr = scrp.tile([CH, T], F32, tag="scr", name=f"scr{c}")
                    nc.gpsimd.dma_start(
                        scr[:, 0:jcap],
                        row[0:1, :w].rearrange("p (t j) -> p t j", j=jcap),
                    )
                    nc.gpsimd.dma_start(
                        scores[tp0 : tp0 + CH, tc_i, 0:jcap], scr[:, 0:jcap]
                    )

                def emit_hs_half(tc_i):
                    for ht in range(2):
                        tr_ps = qk_ps.tile(
                            [P, P], FP16, tag="qkps", name=f"hst{tc_i}{ht}"
                        )
                        nc.tensor.transpose(
                            tr_ps[:],
                            combT[:, 2 + ht, tc_i * P : (tc_i + 1) * P],
                            ident16[:],
                        )
                        nc.vector.tensor_copy(
                            hs[:, tc_i, ht * P : (ht + 1) * P], tr_ps[:]
                        )

                def emit_softmax_dve_half(tc_i):
                    sl = scores[:, tc_i, :]
                    nc.vector.tensor_tensor(
                        sl, sl, maskadd[:, tc_i, :], mybir.AluOpType.add
                    )
                    nc.scalar.activation(sl, sl, AF.Exp)
                    nc.vector.reduce_sum(
                        ssum[:, tc_i : tc_i + 1], sl, axis=mybir.AxisListType.X
                    )
                    nc.vector.reciprocal(
                        srecip[:, tc_i : tc_i + 1], ssum[:, tc_i : tc_i + 1]
                    )
                    nc.vector.tensor_tensor(
                        sl,
                        sl,
                        srecip[:, tc_i : tc_i + 1].to_broadcast([P, T]),
                        mybir.AluOpType.mult,
                    )

                def emit_softmax_pe_half(tc_i):
                    njt = tc_i + 1  # half 0 only attends j < 128
                    for jt in range(njt):
                        tr_ps = qk_ps.tile(
                            [P, P], F32, tag="qkps", name=f"atr{tc_i}{jt}"
                        )
                        nc.tensor.transpose(
                            tr_ps[:], scores[:, tc_i, jt * P : (jt + 1) * P], ident[:]
                        )
                        nc.vector.tensor_copy(
                            alphaT[:, jt, tc_i * P : (tc_i + 1) * P], tr_ps[:]
                        )

                def emit_context_half(tc_i):
                    """context^T[h, tc-half] = Hs^T @ alpha^T (lhsT = Hs[j, h]).
                    Half 0 rows only attend j < 128, so jt=0 only there."""
                    njt = tc_i + 1
                    for mt in range(2):
                        ps = qk_ps.tile([P, P], F32, tag="qkps", name=f"ctx{tc_i}{mt}")
                        for jt in range(njt):
                            nc.tensor.matmul(
                                ps[:],
                                hs[:, jt, mt * P : (mt + 1) * P],
                                alphaT[:, jt, tc_i * P : (tc_i + 1) * P],
                                start=(jt == 0),
                                stop=(jt == njt - 1),
                            )
                        nc.scalar.activation(
                            combT[:, mt, tc_i * P : (tc_i + 1) * P], ps[:], AF.Copy
                        )
                    if tc_i == 0:
                        # t=0 has no past: zero the context column
                        nc.gpsimd.memset(combT[:, 0:2, 0:1], 0.0)
                    nc.vector.tensor_copy(
                        combTr[:, :, tc_i * P : (tc_i + 1) * P],
                        combT[:, :, tc_i * P : (tc_i + 1) * P],
                    )

                def emit_outproj(s, mts, wsl):
                    """out rows [mt*128,(mt+1)*128) for vocab slab s, for each
                    mt in mts, using the already-loaded W_out slab tile."""
                    n0 = s * NS
                    nsub = NS // SUB
                    if not zb:
                        bsl = bp.tile(
                            [1, NS], BF16, tag="bslab", name=f"bsl{s}_{mts[0]}"
                        )
                        nc.sync.dma_start(bsl[:], bout_d[:, n0 : n0 + NS])
                    for mt in mts:
                        osb = op.tile([P, NS], FP16, tag="osb", name=f"osb{s}_{mt}")
                        pss = [
                            pg_ps.tile(
                                [P, SUB], F32, tag="ops", name=f"ops{s}_{mt}{i}"
                            )
                            for i in range(nsub)
                        ]
                        for kt in range(4):
                            for i in range(nsub):
                                nc.tensor.matmul(
                                    pss[i][:],
                                    combTr[:, kt, mt * P : (mt + 1) * P],
                                    wsl[:, kt, i * SUB : (i + 1) * SUB],
                                    start=(kt == 0),
                                    stop=(zb and kt == 3),
                                )
                        if not zb:
                            for i in range(nsub):
                                nc.tensor.matmul(
                                    pss[i][:],
                                    ones_row[:],
                                    bsl[:, i * SUB : (i + 1) * SUB],
                                    start=False,
                                    stop=True,
                                )
                        for i in range(nsub):
                            if (s + mt) % 2 == 1:
                                nc.vector.tensor_copy(
                                    osb[:, i * SUB : (i + 1) * SUB], pss[i][:]
                                )
                            else:
                                nc.scalar.activation(
                                    osb[:, i * SUB : (i + 1) * SUB], pss[i][:], AF.Copy
                                )
                        nc.sync.dma_start(
                            out_d[mt * P : (mt + 1) * P, n0 : n0 + NS], osb[:]
                        )

                nc.gpsimd.memset(scores[:], 0.0)

                # Batched x-inject: ONE matmul writes all T x-columns for BOTH
                # mt halves into the persistent scan PSUM bank (a start=True
                # matmul clears has_written for the WHOLE bank, so both halves
                # must be covered by one start=True write). The per-step W_hh
                # matmuls then accumulate into their column (start=False).
                sc = sc_ps.tile([P, 2, T], F32, tag="scanps", name="scanps")
                nc.tensor.matmul(
                    sc[:, :, :],
                    ident16[:],
                    xT16[:, :, :],
                    start=True,
                    stop=False,
                    skip_group_check=True,
                )
                # t = 0: h_0 = tanh(x_0)
                nc.scalar.activation(combT[:, 2:4, 0:1], sc[:, :, 0:1], AF.Tanh)

                # in-scan emission schedule:
                #  - qk block b (b<3): 2-matmul pieces at steps 64(b+1)+1..+8,
                #    DVE copies at +10 (PE-slack sized, no chain impact)
                #  - held-chunk energy adds (DVE only, idle during scan)
                qk_piece_step = {}
                qk_copy_step = {}
                for b_ in range(3):
                    for g_ in range(4):
                        qk_piece_step[QKB * (b_ + 1) + 1 + g_] = (b_, g_)
                    qk_copy_step[QKB * (b_ + 1) + 6] = b_
                energy_step = {}
                for c in range(HOLD):
                    blk = (CH * c + CH - 1) // QKB
                    t_c = QKB * (blk + 1) + 8 + 2 * (c - (QKB // CH) * blk)
                    energy_step.setdefault(t_c, []).append(c)

                for t in range(1, T):
                    for mt in range(2):
                        for kt in range(2):
                            nc.tensor.matmul(
                                sc[:, mt, t : t + 1],
                                whh[:, kt, mt * P : (mt + 1) * P],
                                combT[:, 2 + kt, t - 1 : t],
                                start=False,
                                stop=(kt == 1),
                                skip_group_check=True,
                            )
                    nc.scalar.activation(
                        combT[:, 2:4, t : t + 1], sc[:, :, t : t + 1], AF.Tanh
                    )
                    if t in qk_piece_step:
                        emit_qk_piece(*qk_piece_step[t])
                    if t in qk_copy_step:
                        emit_qk_copies(qk_copy_step[t])
                    if t == 130:
                        emit_hs_half(0)
                    for c in energy_step.get(t, []):
                        emit_chunk_energy(c)

                # --- post-scan phase ---
                for g_ in range(4):
                    emit_qk_piece(3, g_)
                emit_qk_copies(3)
                emit_hs_half(1)

                # burst: tanh + v-reduce for the held chunks (energies ready)
                LAG = 2
                for c in range(HOLD):
                    emit_chunk_tanh(c)
                    if c >= LAG:
                        emit_chunk_reduce(c - LAG)
                for c in range(HOLD - LAG, HOLD):
                    emit_chunk_reduce(c)

                emit_softmax_dve_half(0)
                emit_softmax_pe_half(0)
                emit_context_half(0)

                # stream remaining chunks, interleaved with mt=0 out-proj on
                # the persistent slabs
                pre_slab = 0
                for c in range(HOLD, n_chunks):
                    emit_chunk_energy(c)
                    emit_chunk_tanh(c)
                    if c - LAG >= HOLD:
                        emit_chunk_reduce(c - LAG)
                    if pre_slab < NPRE:
                        emit_outproj(pre_slab, [0], wsl_tiles[pre_slab])
                        pre_slab += 1
                for c in range(n_chunks - LAG, n_chunks):
                    emit_chunk_reduce(c)
                while pre_slab < NPRE:
                    emit_outproj(pre_slab, [0], wsl_tiles[pre_slab])
                    pre_slab += 1

                # mt=0 for the ring slabs (re-DMAed later for mt=1)
                n_slabs = V // NS
                with tc.tile_pool(name="wpool2", bufs=3) as wp2:
                    for s in range(NPRE, n_slabs):
                        wsl = wp2.tile([P, 4, NS], BF16, tag="wslab2")
                        nc.sync.dma_start(
                            wsl[:], wout_d[:, :, s * NS : (s + 1) * NS]
                        )
                        emit_outproj(s, [0], wsl)

                    emit_softmax_dve_half(1)
                    emit_softmax_pe_half(1)
                    emit_context_half(1)

                    # mt=1 pass: ring slabs re-DMAed, persistent slabs reused
                    for s in range(NPRE, n_slabs):
                        wsl = wp2.tile([P, 4, NS], BF16, tag="wslab2")
                        nc.sync.dma_start(
                            wsl[:], wout_d[:, :, s * NS : (s + 1) * NS]
                        )
                        emit_outproj(s, [1], wsl)
                    for s in range(NPRE):
                        emit_outproj(s, [1], wsl_tiles[s])

            wp_cm.__exit__(None, None, None)

    nc.compile()
    return nc


_NC_CACHE = {}


def _get_nc(zb):
    key = ("nc", zb)
    if key not in _NC_CACHE:
        _NC_CACHE[key] = build_nc(zb=zb)
    return _NC_CACHE[key]


def _prep(inputs):
    input = np.asarray(inputs["input"])
    embedding = np.ascontiguousarray(np.asarray(inputs["embedding"], np.float32))
    W_ih, b_ih = inputs["W_ih"], inputs["b_ih"]
    W_hh, b_hh = inputs["W_hh"], inputs["b_hh"]
    W_ac, b_ac = inputs["W_ac"], inputs["b_ac"]
    W_ap, b_ap = inputs["W_ap"], inputs["b_ap"]
    v_attn, W_out, b_out = inputs["v_attn"], inputs["W_out"], inputs["b_out"]
    zb = bool(
        not np.any(b_ih)
        and not np.any(b_hh)
        and not np.any(b_ac)
        and not np.any(b_ap)
        and not np.any(b_out)
    )

    t_idx = np.arange(T)
    j_idx = np.arange(T)
    maskadd = np.where(
        j_idx[None, :] < (t_idx[:, None]), 0.0, -1e9
    ).astype(np.float32)  # [t, j]
    maskadd = np.ascontiguousarray(
        maskadd.reshape(2, P, T).transpose(1, 0, 2)
    )  # [tp, tc, j]

    import ml_dtypes

    wout_r = np.ascontiguousarray(
        np.asarray(W_out, np.float32)
        .astype(ml_dtypes.bfloat16)
        .reshape(4, P, V)
        .transpose(1, 0, 2)
    )
    shared = {
        "emb": embedding,
        "wih": _r2(np.asarray(W_ih, np.float32)),
        "whh": _r2(np.asarray(W_hh, np.float32).astype(SCAN_NP)),
        "wac": _r2(np.asarray(W_ac, np.float32).astype(np.float16)),
        "wap": _r2(np.asarray(W_ap, np.float32).astype(np.float16)),
        "bih": _col(np.asarray(b_ih, np.float32)),
        "bhh": _col(np.asarray(b_hh, np.float32)),
        "bac": _col(np.asarray(b_ac, np.float32)),
        "bap": _col(np.asarray(b_ap, np.float32)),
        "vcol": _col(np.asarray(v_attn, np.float32).astype(np.float16)),
        "maskadd": maskadd,
        "wout": wout_r,
        "bout": np.ascontiguousarray(
            np.asarray(b_out, np.float32).astype(ml_dtypes.bfloat16)[None, :]
        ),
        "ones": np.ones((1, P), ml_dtypes.bfloat16),
    }
    in_maps = []
    for b in range(B):
        m = dict(shared)
        m["idx"] = np.ascontiguousarray(
            input[b].reshape(2, P).T.astype(np.int32)
        )
        in_maps.append(m)

    return in_maps, zb


def _run(inputs, trace=False):
    in_maps, zb = _prep(inputs)
    nc = _get_nc(zb)
    res = run_bass_kernel_spmd(nc, in_maps, list(range(NCORE)), trace=trace)
    out = np.stack([res.results[c]["out"] for c in range(NCORE)], axis=0)
    return np.ascontiguousarray(out.astype(np.float32)), res.exec_time_ns


def kernel(**inputs):
    return _run(inputs)[0]
